# revision 7
# baseline (speedup 1.0000x reference)
"""Contrast-depth MSE loss on 8 Trainium2 NeuronCores.

Math: with d = out - label (per image, 32x32 grid flattened to p in [0,1024)),
the loss is an exact quadratic form

    loss = sum_{p,q} C[p,q] * G[p,q] / (B*8*30*30),
    G[p,q] = sum_img d[img,p] * d[img,q]

where C (the contrast-depth-conv quadratic form) is supported on the
diagonals q-p in {0, +-1, +-31, +-32, +-33}.  Each core computes banded
Gram blocks G[128k+r, 128k+c] (c in [0,161)) on the TensorEngine with
PSUM accumulation over its 2048-image shard; the host applies the C
weights to the diagonals and reduces across cores.

Scheduling: images are laid on SBUF partitions unevenly to balance the
16 SDMA engines (engine 15, serving partitions 92-95 and 124-127, is
~18% slower than the rest, so those partitions carry 12 image-slots
instead of 16 and partitions 0-31 carry 17).  The host pre-arranges
each shard partition-major per chunk so every DMA is a large
contiguous-per-partition rectangle (16KB descriptors).  Chunks are
ramped: small first chunk for an early vector start, 1.5-2MB chunks
mid-stream, and the tiny 32-partition 17th-slot chunk last so the tail
after the final byte is one cheap subtract + 8 matmuls.  PSUM->SBUF
copies are split across the ACT and DVE engines and the result is
written out in two slices.
"""

import numpy as np

_B = 16384
_H = 32
_W = 32
_P = _H * _W  # 1024 pixels
_NCORES = 8
_BSH = _B // _NCORES  # 2048 images per core
_TILE = 128
_BAND = 161  # 128 + max diagonal offset (33)
_NSLOT = 17  # max image-slots per partition
_FREE = _NSLOT * _P  # 17408 f32 per partition


def _block_ncols(k: int) -> int:
    return min(_BAND, _P - 128 * k)


_GRAM_COLS = sum(_block_ncols(k) for k in range(8))  # 7*161 + 128 = 1255


def _build_weights() -> np.ndarray:
    """[128, _GRAM_COLS] weights s.t. loss_sum = sum(W * gram_blocks)."""
    C = np.zeros((_P, _P), dtype=np.float64)
    offs = [(a, b) for a in range(3) for b in range(3) if (a, b) != (1, 1)]
    for a, b in offs:
        for i in range(_H - 2):
            for j in range(_W - 2):
                p = (i + a) * _W + (j + b)  # neighbor pixel
                q = (i + 1) * _W + (j + 1)  # center pixel
                C[p, p] += 1.0
                C[q, q] += 1.0
                C[p, q] -= 1.0
                C[q, p] -= 1.0
    W = np.zeros((_TILE, _GRAM_COLS), dtype=np.float64)
    off = 0
    for k in range(8):
        ncols = _block_ncols(k)
        for delta in (0, 1, 31, 32, 33):
            for r in range(_TILE):
                p = 128 * k + r
                q = p + delta
                c = r + delta
                if q >= _P or c >= ncols:
                    continue
                W[r, off + c] = C[p, q] * (1.0 if delta == 0 else 2.0)
        off += ncols
    return W


_WFULL = _build_weights()

# chunk table: (slot0, nslots, p0, p1).  Slots 0-11 exist on all 128
# partitions, 12-15 on partitions [0,92)+[96,124), 16 on [0,32).  Each
# chunk is DMA'd as one [p1-p0, nslots*1024] rectangle whose source rows
# are partition-major, so every partition reads one contiguous run.
_CHUNKS = [
    (0, 1, 0, 128),
    (1, 4, 0, 128),
    (5, 4, 0, 128),
    (9, 3, 0, 128),
    (12, 4, 0, 92),
    (12, 4, 96, 124),
    (16, 1, 0, 32),
]


def _pack_shard(x: np.ndarray) -> np.ndarray:
    """[2048, 1024] images -> [128, 17408] partition-major chunk layout."""
    packed = np.empty((_TILE, _FREE), dtype=np.float32)
    row = 0
    for s0, ns, p0, p1 in _CHUNKS:
        npart = p1 - p0
        n = npart * ns
        blk = x[row : row + n].reshape(npart, ns * _P)
        packed[p0:p1, s0 * _P : (s0 + ns) * _P] = blk
        row += n
    assert row == _BSH
    return packed


_NC_CACHE = None


def _build_nc():
    import concourse.bacc as bacc
    import concourse.mybir as mybir
    import concourse.tile as tile

    nc = bacc.Bacc()
    out_d = nc.dram_tensor("out", [_TILE, _FREE], mybir.dt.float32, kind="ExternalInput")
    lab_d = nc.dram_tensor("label", [_TILE, _FREE], mybir.dt.float32, kind="ExternalInput")
    gram_d = nc.dram_tensor(
        "gram", [_TILE, _GRAM_COLS], mybir.dt.float32, kind="ExternalOutput"
    )

    with tile.TileContext(nc) as tc:
        with (
            tc.tile_pool(name="buf", bufs=1) as buf_pool,
            tc.tile_pool(name="ps", bufs=1, space="PSUM") as psum_pool,
        ):
            grams = []
            offs = []
            off = 0
            for k in range(8):
                ncols = _block_ncols(k)
                grams.append(
                    psum_pool.tile(
                        [_TILE, ncols], mybir.dt.float32, tag=f"g{k}", name=f"g{k}"
                    )
                )
                offs.append(off)
                off += ncols

            # persistent SBUF buffers: every chunk DMA can enqueue
            # immediately; no pool-slot rotation ever blocks the DMA stream.
            o = buf_pool.tile([_TILE, _FREE], mybir.dt.float32, tag="o", name="o")
            lb = buf_pool.tile([_TILE, _FREE], mybir.dt.float32, tag="l", name="l")
            d = buf_pool.tile([_TILE, _FREE], mybir.dt.bfloat16, tag="d", name="d")
            result = buf_pool.tile(
                [_TILE, _GRAM_COLS], mybir.dt.float32, tag="r", name="r"
            )

            # matmul base partition must be in {0,32,64}: slots 12-15
            # contract over all 128 partitions, so zero the unloaded hole
            # partitions (92-95, 124-127) once.  Runs during the DMA ramp.
            nc.vector.memset(d[:, 12 * _P : 16 * _P], 0.0)

            def emit_mms(slot, mp1, start, stop):
                c0 = slot * _P
                for k in range(8):
                    ncols = _block_ncols(k)
                    nc.tensor.matmul(
                        grams[k][:, :ncols],
                        lhsT=d[0:mp1, c0 + 128 * k : c0 + 128 * k + 128],
                        rhs=d[0:mp1, c0 + 128 * k : c0 + 128 * k + ncols],
                        start=start,
                        stop=stop,
                    )

            for ci, (s0, ns, p0, p1) in enumerate(_CHUNKS):
                c0, c1 = s0 * _P, (s0 + ns) * _P
                if p1 - p0 == 128:
                    # full-width chunks: HWDGE splits them evenly across
                    # all 16 SDMA engines
                    nc.sync.dma_start(out=o[p0:p1, c0:c1], in_=out_d[p0:p1, c0:c1])
                    nc.scalar.dma_start(out=lb[p0:p1, c0:c1], in_=lab_d[p0:p1, c0:c1])
                else:
                    # partial-width chunks: HWDGE concentrates these on ~4
                    # engines; SWDGE assigns by partition ownership, which
                    # spreads them across every engine except slow engine
                    # 15 (whose partitions 92-95/124-127 carry fewer slots)
                    nc.gpsimd.dma_start(out=o[p0:p1, c0:c1], in_=out_d[p0:p1, c0:c1])
                    nc.gpsimd.dma_start(out=lb[p0:p1, c0:c1], in_=lab_d[p0:p1, c0:c1])
                last = ci == len(_CHUNKS) - 1
                if not last:
                    for s in range(s0, s0 + ns):
                        w0, w1 = s * _P, (s + 1) * _P
                        nc.vector.tensor_sub(
                            out=d[p0:p1, w0:w1], in0=o[p0:p1, w0:w1], in1=lb[p0:p1, w0:w1]
                        )
                        if p0 == 0 and p1 == 128:
                            emit_mms(s, 128, start=(s == 0), stop=False)
                    if (s0, p0) == (12, 96):
                        # both halves of slots 12-15 are now in d
                        for s in range(12, 16):
                            emit_mms(s, 128, start=False, stop=False)
                else:
                    # tail chunk (slot 16, partitions 0-32): split the
                    # subtract at the block 4/5 boundary so matmuls and
                    # PSUM->SBUF copies start before it finishes.
                    sp = 673
                    nc.vector.tensor_sub(
                        out=d[0:32, c0 : c0 + sp],
                        in0=o[0:32, c0 : c0 + sp],
                        in1=lb[0:32, c0 : c0 + sp],
                    )
                    nc.vector.tensor_sub(
                        out=d[0:32, c0 + sp : c1],
                        in0=o[0:32, c0 + sp : c1],
                        in1=lb[0:32, c0 + sp : c1],
                    )
                    emit_mms(16, 32, start=False, stop=True)

            # PSUM -> SBUF: blocks 0-3 on the ACT engine, 4-7 on DVE (which
            # is finishing the tail subtract), then 2 output DMA slices.
            for k in range(8):
                ncols = _block_ncols(k)
                dst = result[:, offs[k] : offs[k] + ncols]
                if k < 4:
                    nc.scalar.copy(out=dst, in_=grams[k][:])
                else:
                    nc.vector.tensor_copy(out=dst, in_=grams[k][:])
            split = offs[4]
            nc.sync.dma_start(out=gram_d[:, :split], in_=result[:, :split])
            nc.sync.dma_start(out=gram_d[:, split:], in_=result[:, split:])
    nc.finalize()
    return nc


def _run(out, label, trace=False):
    from concourse.bass_utils import run_bass_kernel_spmd

    global _NC_CACHE
    out = np.asarray(out, dtype=np.float32).reshape(_B, _P)
    label = np.asarray(label, dtype=np.float32).reshape(_B, _P)
    if _NC_CACHE is None:
        _NC_CACHE = _build_nc()
    in_maps = [
        {
            "out": _pack_shard(out[i * _BSH : (i + 1) * _BSH]),
            "label": _pack_shard(label[i * _BSH : (i + 1) * _BSH]),
        }
        for i in range(_NCORES)
    ]
    res = run_bass_kernel_spmd(
        _NC_CACHE, in_maps, core_ids=list(range(_NCORES)), trace=trace
    )
    total = 0.0
    for r in res.results:
        total += float((_WFULL * r["gram"].astype(np.float64)).sum())
    loss = total / (_B * 8 * (_H - 2) * (_W - 2))
    return np.asarray(np.float32(loss)), res


def kernel(out, label):
    loss, _ = _run(out, label, trace=False)
    return loss


# revision 8
# speedup vs baseline: 1.5219x; 1.5219x over previous
"""Contrast-depth MSE loss on 8 Trainium2 NeuronCores.

Math: with d = out - label (per image, 32x32 grid flattened to p in [0,1024)),
the loss is an exact quadratic form

    loss = sum_{p,q} C[p,q] * G[p,q] / (B*8*30*30),
    G[p,q] = sum_img d[img,p] * d[img,q]

where C (the contrast-depth-conv quadratic form) is supported on the
diagonals q-p in {0, +-1, +-31, +-32, +-33}.  Each core computes banded
Gram blocks G[128k+r, 128k+c] (c in [0,161)) on the TensorEngine with
PSUM accumulation over its 2048-image shard; the host applies the C
weights to the diagonals and reduces across cores.

Scheduling: the host shard [2048, 1024] is viewed as [128, 16*1024]
(partition p holds images 16p..16p+15 -- a free reshape), so every DMA
is a full-width rectangle with long contiguous per-partition runs,
which HWDGE splits evenly across all 16 SDMA engines.  Chunks are
ramped [1,4,4,4,2,1] image-slots: a small first chunk starts the
vector/tensor pipeline early, 2MB chunks saturate mid-stream, and a
small last chunk keeps the tail short.  The final slot's subtract is
split at the gram-block 4/5 boundary so matmuls and PSUM->SBUF copies
(split across the ACT and DVE engines) overlap it, and the result goes
out in two DMA slices.
"""

import numpy as np

_B = 16384
_H = 32
_W = 32
_P = _H * _W  # 1024 pixels
_NCORES = 8
_BSH = _B // _NCORES  # 2048 images per core
_TILE = 128
_BAND = 161  # 128 + max diagonal offset (33)
_NSLOT = 16  # image-slots per partition
_FREE = _NSLOT * _P  # 16384 f32 per partition


def _block_ncols(k: int) -> int:
    return min(_BAND, _P - 128 * k)


_GRAM_COLS = sum(_block_ncols(k) for k in range(8))  # 7*161 + 128 = 1255


def _build_weights() -> np.ndarray:
    """[128, _GRAM_COLS] weights s.t. loss_sum = sum(W * gram_blocks)."""
    C = np.zeros((_P, _P), dtype=np.float64)
    offs = [(a, b) for a in range(3) for b in range(3) if (a, b) != (1, 1)]
    for a, b in offs:
        for i in range(_H - 2):
            for j in range(_W - 2):
                p = (i + a) * _W + (j + b)  # neighbor pixel
                q = (i + 1) * _W + (j + 1)  # center pixel
                C[p, p] += 1.0
                C[q, q] += 1.0
                C[p, q] -= 1.0
                C[q, p] -= 1.0
    W = np.zeros((_TILE, _GRAM_COLS), dtype=np.float64)
    off = 0
    for k in range(8):
        ncols = _block_ncols(k)
        for delta in (0, 1, 31, 32, 33):
            for r in range(_TILE):
                p = 128 * k + r
                q = p + delta
                c = r + delta
                if q >= _P or c >= ncols:
                    continue
                W[r, off + c] = C[p, q] * (1.0 if delta == 0 else 2.0)
        off += ncols
    return W


_WFULL = _build_weights()

# ramped chunk sizes in image-slots: small at both ends (early pipeline
# start, short tail), 2MB chunks mid-stream
_CHUNKS = [1, 4, 4, 4, 2, 1]
assert sum(_CHUNKS) == _NSLOT

_NC_CACHE = None


def _build_nc():
    import concourse.bacc as bacc
    import concourse.mybir as mybir
    import concourse.tile as tile

    nc = bacc.Bacc()
    out_d = nc.dram_tensor("out", [_TILE, _FREE], mybir.dt.float32, kind="ExternalInput")
    lab_d = nc.dram_tensor("label", [_TILE, _FREE], mybir.dt.float32, kind="ExternalInput")
    gram_d = nc.dram_tensor(
        "gram", [_TILE, _GRAM_COLS], mybir.dt.float32, kind="ExternalOutput"
    )

    with tile.TileContext(nc) as tc:
        with (
            tc.tile_pool(name="buf", bufs=1) as buf_pool,
            tc.tile_pool(name="ps", bufs=1, space="PSUM") as psum_pool,
        ):
            grams = []
            offs = []
            off = 0
            for k in range(8):
                ncols = _block_ncols(k)
                grams.append(
                    psum_pool.tile(
                        [_TILE, ncols], mybir.dt.float32, tag=f"g{k}", name=f"g{k}"
                    )
                )
                offs.append(off)
                off += ncols

            # persistent SBUF buffers: every chunk DMA can enqueue
            # immediately; no pool-slot rotation ever blocks the DMA stream.
            o = buf_pool.tile([_TILE, _FREE], mybir.dt.float32, tag="o", name="o")
            lb = buf_pool.tile([_TILE, _FREE], mybir.dt.float32, tag="l", name="l")
            d = buf_pool.tile([_TILE, _FREE], mybir.dt.bfloat16, tag="d", name="d")
            result = buf_pool.tile(
                [_TILE, _GRAM_COLS], mybir.dt.float32, tag="r", name="r"
            )

            def emit_mms(slot, start, stop):
                c0 = slot * _P
                for k in range(8):
                    ncols = _block_ncols(k)
                    nc.tensor.matmul(
                        grams[k][:, :ncols],
                        lhsT=d[:, c0 + 128 * k : c0 + 128 * k + 128],
                        rhs=d[:, c0 + 128 * k : c0 + 128 * k + ncols],
                        start=start,
                        stop=stop,
                    )

            s0 = 0
            for ci, ns in enumerate(_CHUNKS):
                c0, c1 = s0 * _P, (s0 + ns) * _P
                nc.sync.dma_start(out=o[:, c0:c1], in_=out_d[:, c0:c1])
                nc.scalar.dma_start(out=lb[:, c0:c1], in_=lab_d[:, c0:c1])
                last = ci == len(_CHUNKS) - 1
                if not last:
                    for s in range(s0, s0 + ns):
                        w0, w1 = s * _P, (s + 1) * _P
                        nc.vector.tensor_sub(
                            out=d[:, w0:w1], in0=o[:, w0:w1], in1=lb[:, w0:w1]
                        )
                        emit_mms(s, start=(s == 0), stop=False)
                else:
                    # tail slot: split the subtract at the block 4/5
                    # boundary so matmuls and PSUM->SBUF copies start
                    # before it finishes.
                    sp = 673
                    nc.vector.tensor_sub(
                        out=d[:, c0 : c0 + sp],
                        in0=o[:, c0 : c0 + sp],
                        in1=lb[:, c0 : c0 + sp],
                    )
                    nc.vector.tensor_sub(
                        out=d[:, c0 + sp : c1],
                        in0=o[:, c0 + sp : c1],
                        in1=lb[:, c0 + sp : c1],
                    )
                    emit_mms(s0, start=False, stop=True)
                s0 += ns

            # PSUM -> SBUF: blocks 0-3 on the ACT engine, 4-7 on DVE (which
            # is finishing the tail subtract), then 2 output DMA slices.
            for k in range(8):
                ncols = _block_ncols(k)
                dst = result[:, offs[k] : offs[k] + ncols]
                if k < 4:
                    nc.scalar.copy(out=dst, in_=grams[k][:])
                else:
                    nc.vector.tensor_copy(out=dst, in_=grams[k][:])
            split = offs[4]
            nc.sync.dma_start(out=gram_d[:, :split], in_=result[:, :split])
            nc.sync.dma_start(out=gram_d[:, split:], in_=result[:, split:])
    nc.finalize()
    return nc


def _run(out, label, trace=False):
    from concourse.bass_utils import run_bass_kernel_spmd

    global _NC_CACHE
    out = np.ascontiguousarray(np.asarray(out), dtype=np.float32).reshape(_B, _P)
    label = np.ascontiguousarray(np.asarray(label), dtype=np.float32).reshape(_B, _P)
    if _NC_CACHE is None:
        _NC_CACHE = _build_nc()
    in_maps = [
        {
            "out": out[i * _BSH : (i + 1) * _BSH],
            "label": label[i * _BSH : (i + 1) * _BSH],
        }
        for i in range(_NCORES)
    ]
    res = run_bass_kernel_spmd(
        _NC_CACHE, in_maps, core_ids=list(range(_NCORES)), trace=trace
    )
    total = 0.0
    for r in res.results:
        total += float((_WFULL * r["gram"].astype(np.float64)).sum())
    loss = total / (_B * 8 * (_H - 2) * (_W - 2))
    return np.asarray(np.float32(loss)), res


def kernel(out, label):
    loss, _ = _run(out, label, trace=False)
    return loss


# revision 9
# speedup vs baseline: 1.5740x; 1.0343x over previous
"""Contrast-depth MSE loss on 8 Trainium2 NeuronCores.

Math: with d = out - label (per image, 32x32 grid flattened to p in [0,1024)),
the loss is an exact quadratic form

    loss = sum_{p,q} C[p,q] * G[p,q] / (B*8*30*30),
    G[p,q] = sum_img d[img,p] * d[img,q]

where C (the contrast-depth-conv quadratic form) is supported on the
diagonals q-p in {0, +-1, +-31, +-32, +-33}.  Each core computes banded
Gram blocks G[128k+r, 128k+c] (c in [0,161)) on the TensorEngine with
PSUM accumulation over its 2048-image shard; the host applies the C
weights to the diagonals and reduces across cores.

Scheduling: HWDGE splits a [p0:p1, :] DMA across the largest divisor of
(p1-p0) that is <= 16 SDMA engines, assigned positionally from engine 0.
SDMA engine 15 is ~20% slower than the rest, so the layout is tilted:
13 image-slots are full-width (engine 15 carries exactly those), 3 extra
slots ride on [0:120] chunks that split across engines 0-14 only, and a
last 24-image slot goes on a [0:24] chunk (engines 0-11).  All engines
then finish the stream together.  The host packs each shard
partition-major per chunk so every DMA reads long contiguous runs.
Chunks are ramped (small first chunk for an early vector start, 2MB
mid-stream, 0.5MB single-slot chunks at the end) so the tail after the
final byte is one split subtract + 8 matmuls overlapped with PSUM->SBUF
copies on the ACT and DVE engines, then 2 output DMA slices.
"""

import numpy as np

_B = 16384
_H = 32
_W = 32
_P = _H * _W  # 1024 pixels
_NCORES = 8
_BSH = _B // _NCORES  # 2048 images per core
_TILE = 128
_BAND = 161  # 128 + max diagonal offset (33)
_NSLOT = 17  # max image-slots per partition
_FREE = _NSLOT * _P


def _block_ncols(k: int) -> int:
    return min(_BAND, _P - 128 * k)


_GRAM_COLS = sum(_block_ncols(k) for k in range(8))  # 7*161 + 128 = 1255


def _build_weights() -> np.ndarray:
    """[128, _GRAM_COLS] weights s.t. loss_sum = sum(W * gram_blocks)."""
    C = np.zeros((_P, _P), dtype=np.float64)
    offs = [(a, b) for a in range(3) for b in range(3) if (a, b) != (1, 1)]
    for a, b in offs:
        for i in range(_H - 2):
            for j in range(_W - 2):
                p = (i + a) * _W + (j + b)  # neighbor pixel
                q = (i + 1) * _W + (j + 1)  # center pixel
                C[p, p] += 1.0
                C[q, q] += 1.0
                C[p, q] -= 1.0
                C[q, p] -= 1.0
    W = np.zeros((_TILE, _GRAM_COLS), dtype=np.float64)
    off = 0
    for k in range(8):
        ncols = _block_ncols(k)
        for delta in (0, 1, 31, 32, 33):
            for r in range(_TILE):
                p = 128 * k + r
                q = p + delta
                c = r + delta
                if q >= _P or c >= ncols:
                    continue
                W[r, off + c] = C[p, q] * (1.0 if delta == 0 else 2.0)
        off += ncols
    return W


_WFULL = _build_weights()

# chunk table in DMA order: (npart, slot0, nslots).  Slots 0-12 exist on
# all 128 partitions, 13-15 on partitions [0,120), 16 on [0,24).
# 13*128 + 24 + 3*120 = 2048 images.
_CHUNKS = [
    (128, 0, 1),
    (128, 1, 4),
    (128, 5, 4),
    (128, 9, 4),
    (24, 16, 1),
    (120, 13, 1),
    (120, 14, 1),
    (120, 15, 1),
]
assert sum(np * ns for np, _, ns in _CHUNKS) == _BSH

# per-slot compute order (the last one is the tail tile); (slot, npart)
_SLOT_ORDER = (
    [(0, 128)]
    + [(s, 128) for s in range(1, 13)]
    + [(16, 24), (13, 120), (14, 120), (15, 120)]
)

_NC_CACHE = None


def _pack_shard(x: np.ndarray) -> np.ndarray:
    """[2048, 1024] images -> [128, 17408] partition-major chunk layout."""
    packed = np.empty((_TILE, _FREE), dtype=np.float32)
    row = 0
    for npart, s0, ns in _CHUNKS:
        n = npart * ns
        packed[0:npart, s0 * _P : (s0 + ns) * _P] = x[row : row + n].reshape(
            npart, ns * _P
        )
        row += n
    assert row == _BSH
    return packed


def _build_nc():
    import concourse.bacc as bacc
    import concourse.mybir as mybir
    import concourse.tile as tile

    nc = bacc.Bacc()
    out_d = nc.dram_tensor("out", [_TILE, _FREE], mybir.dt.float32, kind="ExternalInput")
    lab_d = nc.dram_tensor("label", [_TILE, _FREE], mybir.dt.float32, kind="ExternalInput")
    gram_d = nc.dram_tensor(
        "gram", [_TILE, _GRAM_COLS], mybir.dt.float32, kind="ExternalOutput"
    )

    with tile.TileContext(nc) as tc:
        with (
            tc.tile_pool(name="buf", bufs=1) as buf_pool,
            tc.tile_pool(name="ps", bufs=1, space="PSUM") as psum_pool,
        ):
            grams = []
            offs = []
            off = 0
            for k in range(8):
                ncols = _block_ncols(k)
                grams.append(
                    psum_pool.tile(
                        [_TILE, ncols], mybir.dt.float32, tag=f"g{k}", name=f"g{k}"
                    )
                )
                offs.append(off)
                off += ncols

            # persistent SBUF buffers: every chunk DMA can enqueue
            # immediately; no pool-slot rotation ever blocks the DMA stream.
            o = buf_pool.tile([_TILE, _FREE], mybir.dt.float32, tag="o", name="o")
            lb = buf_pool.tile([_TILE, _FREE], mybir.dt.float32, tag="l", name="l")
            d = buf_pool.tile([_TILE, _FREE], mybir.dt.bfloat16, tag="d", name="d")
            result = buf_pool.tile(
                [_TILE, _GRAM_COLS], mybir.dt.float32, tag="r", name="r"
            )

            for npart, s0, ns in _CHUNKS:
                c0, c1 = s0 * _P, (s0 + ns) * _P
                nc.sync.dma_start(out=o[0:npart, c0:c1], in_=out_d[0:npart, c0:c1])
                nc.scalar.dma_start(out=lb[0:npart, c0:c1], in_=lab_d[0:npart, c0:c1])

            def emit_mms(slot, npart, start, stop):
                c0 = slot * _P
                for k in range(8):
                    ncols = _block_ncols(k)
                    nc.tensor.matmul(
                        grams[k][:, :ncols],
                        lhsT=d[0:npart, c0 + 128 * k : c0 + 128 * k + 128],
                        rhs=d[0:npart, c0 + 128 * k : c0 + 128 * k + ncols],
                        start=start,
                        stop=stop,
                    )

            nslots = len(_SLOT_ORDER)
            for si, (s, npart) in enumerate(_SLOT_ORDER):
                c0, c1 = s * _P, (s + 1) * _P
                if si < nslots - 1:
                    nc.vector.tensor_sub(
                        out=d[0:npart, c0:c1], in0=o[0:npart, c0:c1], in1=lb[0:npart, c0:c1]
                    )
                    emit_mms(s, npart, start=(si == 0), stop=False)
                else:
                    # tail slot: split the subtract at the block 4/5
                    # boundary so matmuls and PSUM->SBUF copies start
                    # before it finishes.
                    sp = 673
                    nc.vector.tensor_sub(
                        out=d[0:npart, c0 : c0 + sp],
                        in0=o[0:npart, c0 : c0 + sp],
                        in1=lb[0:npart, c0 : c0 + sp],
                    )
                    nc.vector.tensor_sub(
                        out=d[0:npart, c0 + sp : c1],
                        in0=o[0:npart, c0 + sp : c1],
                        in1=lb[0:npart, c0 + sp : c1],
                    )
                    emit_mms(s, npart, start=False, stop=True)

            # PSUM -> SBUF: blocks 0-3 on the ACT engine, 4-7 on DVE (which
            # is finishing the tail subtract), then 2 output DMA slices.
            for k in range(8):
                ncols = _block_ncols(k)
                dst = result[:, offs[k] : offs[k] + ncols]
                if k < 4:
                    nc.scalar.copy(out=dst, in_=grams[k][:])
                else:
                    nc.vector.tensor_copy(out=dst, in_=grams[k][:])
            split = offs[4]
            nc.sync.dma_start(out=gram_d[:, :split], in_=result[:, :split])
            nc.sync.dma_start(out=gram_d[:, split:], in_=result[:, split:])
    nc.finalize()
    return nc


def _run(out, label, trace=False):
    from concourse.bass_utils import run_bass_kernel_spmd

    global _NC_CACHE
    out = np.asarray(out, dtype=np.float32).reshape(_B, _P)
    label = np.asarray(label, dtype=np.float32).reshape(_B, _P)
    if _NC_CACHE is None:
        _NC_CACHE = _build_nc()
    in_maps = [
        {
            "out": _pack_shard(out[i * _BSH : (i + 1) * _BSH]),
            "label": _pack_shard(label[i * _BSH : (i + 1) * _BSH]),
        }
        for i in range(_NCORES)
    ]
    res = run_bass_kernel_spmd(
        _NC_CACHE, in_maps, core_ids=list(range(_NCORES)), trace=trace
    )
    total = 0.0
    for r in res.results:
        total += float((_WFULL * r["gram"].astype(np.float64)).sum())
    loss = total / (_B * 8 * (_H - 2) * (_W - 2))
    return np.asarray(np.float32(loss)), res


def kernel(out, label):
    loss, _ = _run(out, label, trace=False)
    return loss
